# revision 1
# baseline (speedup 1.0000x reference)
"""Trainium2 Bass kernel for HeatmapMaxDetBlock (argmax + local refinement).

Computes, for x[B, C, H, W]:
    scores = max over (H*W); idx = argmax; px = idx % W, py = idx // W (masked
    by score > 0); quarter-pixel refinement by sign of neighbor differences.
Returns [B, C, 3] = (px, py, scores).

Strategy (pure data parallel over 8 NeuronCores, batch-sharded; 136 heatmap
rows of H*W=49152 f32 per core):
  Rows live DIRECTLY on SBUF partitions. Main group = rows 0..127 on 128
  partitions; the free dim streams the 49152 row columns in chunks (24 KiB
  contiguous per partition per DMA -> near-peak HBM bandwidth). One DVE
  reduce per chunk produces per-(row, segment) maxima with segment width
  W=192, so the winning segment IS py and the in-segment argmax IS px --
  no transpose, no relayout, no integer division.
  The 8 leftover rows (128..135) stream first as a [128, 3072] tile
  (16 partitions per row), get a tiny linearizing SBUF->SBUF DMA into
  [8, 256] segment-max form, and their whole detection chain hides under
  the main stream. Only the main group's short chain (segment argmax ->
  window gather -> max_index -> neighbor gather -> refinement) is exposed
  after the last chunk lands.
"""

import sys
from contextlib import ExitStack
from dataclasses import dataclass

import numpy as np

for _p in ("/opt/trn_rl_repo",):
    if _p not in sys.path:
        sys.path.insert(0, _p)

import concourse.bass as bass  # noqa: E402
import concourse.tile as tile  # noqa: E402
from concourse import bacc, mybir  # noqa: E402

F32 = mybir.dt.float32
I32 = mybir.dt.int32
U32 = mybir.dt.uint32
AX = mybir.AxisListType
OP = mybir.AluOpType


@dataclass(frozen=True)
class Cfg:
    B: int = 64
    C: int = 17
    H: int = 256
    W: int = 192
    ncores: int = 8
    P: int = 128
    FRONT: int = 256
    REAR: int = 512

    @property
    def BP(self):  # batches per core
        return self.B // self.ncores

    @property
    def R(self):  # heatmap rows per core
        return self.BP * self.C

    @property
    def HWm(self):
        return self.H * self.W

    @property
    def NSEG(self):  # segments per row (segment = one heatmap line)
        return self.H

    @property
    def NBW(self):  # neighborhood gather width: [-W .. +W]
        return 2 * self.W + 1

    @property
    def SHN(self):
        return self.R * self.HWm

    @property
    def NPAD(self):
        return self.FRONT + self.SHN + self.REAR

    @property
    def RUMP(self):  # leftover rows beyond the 128-partition main group
        return self.R - self.P


CFG = Cfg()

# Column chunking of the main group's stream. There are 8 HWDGE semaphore
# lanes; a lane's (i+8)-th DMA can only issue after the reduce consuming
# its i-th DMA ran. Nine small chunks up front make all those gates
# resolve early, so the bigger chunks stream without ever waiting on DVE
# progress. Chunks alternate between the two HWDGE queues; both queues
# end with a small chunk so the final deliveries (which gate the argmax
# chain) land early and together.
CHUNKS = [768, 768, 1536, 1536, 3072, 3072, 3072, 3072, 3072] + [
    4224, 4224, 4224, 4224, 4224, 4032, 2496, 768, 768]
assert sum(CHUNKS) == CFG.HWm and all(c % CFG.W == 0 for c in CHUNKS)


def build_program(cfg: Cfg):
    c = cfg
    W = c.W
    assert c.RUMP * 16 == c.P, "rump layout assumes 16 partitions per row"
    assert c.FRONT >= W and c.REAR >= 2 * W

    nc = bacc.Bacc(
        "TRN2", target_bir_lowering=False, debug=False, num_devices=c.ncores
    )
    xh = nc.dram_tensor("x", [c.NPAD], F32, kind="ExternalInput").ap()
    oh = nc.dram_tensor("out", [c.R, 3], F32, kind="ExternalOutput").ap()

    with ExitStack() as ctx:
        tc = ctx.enter_context(tile.TileContext(nc))
        xpool = ctx.enter_context(tc.tile_pool(name="xp", bufs=8))
        sp = ctx.enter_context(tc.tile_pool(name="sp", bufs=1))

        # ---- stream DMAs -------------------------------------------------
        # main-group chunk DMAs, alternating the two HWDGE queues
        xts = []
        col = 0
        for i, ncols in enumerate(CHUNKS):
            xt = xpool.tile([c.P, max(CHUNKS)], F32, tag="xt")
            src = bass.AP(
                xh.tensor, c.FRONT + col, [[c.HWm, c.P], [1, ncols]]
            )
            eng = nc.sync if i % 2 == 0 else nc.scalar
            eng.dma_start(out=xt[:, 0:ncols], in_=src)
            xts.append((xt, col, ncols))
            col += ncols

        # rump rows ride the (otherwise idle) SWDGE queue; their phase 2
        # hides under the main stream
        xtr = sp.tile([c.P, 3072], F32, tag="xtr")
        rsrc = bass.AP(
            xh.tensor,
            c.FRONT + c.P * c.HWm,
            [[c.HWm, c.RUMP], [3072, 16], [1, 3072]],
        )
        nc.gpsimd.dma_start(out=xtr[:], in_=rsrc)

        # ---- on-chip constants (cheap; off the critical path) ------------
        # values stay below 2^24 so f32 iota is exact
        # rowbase[p] = FRONT + row_p*HWm
        rb_g = sp.tile([c.P, 1], F32, tag="rb_g")
        nc.gpsimd.iota(rb_g[:], pattern=[[0, 1]], base=c.FRONT,
                       channel_multiplier=c.HWm,
                       allow_small_or_imprecise_dtypes=True)
        rb_r = sp.tile([c.RUMP, 1], F32, tag="rb_r")
        nc.gpsimd.iota(rb_r[:], pattern=[[0, 1]],
                       base=c.FRONT + c.P * c.HWm, channel_multiplier=c.HWm,
                       allow_small_or_imprecise_dtypes=True)
        # interior upper bounds (px < W-1, py < H-1)
        hi2 = sp.tile([c.P, 2], F32, tag="hi2")
        nc.vector.memset(hi2[:, 0:1], float(W - 1))
        nc.vector.memset(hi2[:, 1:2], float(c.H - 1))

        # ---- phase 1 reduces --------------------------------------------
        Mr = sp.tile([c.P, 16], F32, tag="Mr")
        M = sp.tile([c.P, c.NSEG], F32, tag="M")

        def reduce_chunk(i):
            xt, col, ncols = xts[i]
            s0 = col // W
            ns = ncols // W
            nc.vector.reduce_max(
                out=M[:, s0 : s0 + ns],
                in_=xt[:, 0:ncols].rearrange("p (s u) -> p s u", u=W),
                axis=AX.X,
            )

        # rump relayout: [128,16] partition-major -> [8, 256] rows-on-partitions
        R8 = sp.tile([c.RUMP, c.NSEG], F32, tag="R8")

        # ---- phase 2 chain (per group), in three parts -------------------
        # part 1: segment argmax -> window gather issued
        # part 2: in-window max_index -> neighborhood gather issued
        # part 3: px/py assembly + quarter-pixel refinement
        def chain_p1(Mg, rb, gp, tagp):
            st = {}
            scores = sp.tile([gp, 1], F32, tag=f"sc{tagp}")
            nc.vector.reduce_max(out=scores[:], in_=Mg, axis=AX.X)
            m8 = sp.tile([gp, 8], F32, tag=f"m8{tagp}")
            nc.vector.tensor_copy(out=m8[:], in_=scores[:].to_broadcast([gp, 8]))
            # winning segment = first index of the row max among the
            # segment maxima (exact argmax tie order)
            ms = sp.tile([gp, 8], U32, tag=f"ms{tagp}")
            nc.vector.max_index(ms[:], m8[:], Mg)
            # window start (absolute in padded x): w0 = rb + W*s -- computed
            # straight from the u32 index (casting multiply) so the f32 copy
            # of s stays off the gather's critical path
            w0 = sp.tile([gp, 1], F32, tag=f"w0{tagp}")
            nc.vector.tensor_scalar(
                out=w0[:], in0=ms[:, 0:1], scalar1=float(W), scalar2=None,
                op0=OP.mult,
            )
            nc.vector.tensor_tensor(out=w0[:], in0=w0[:], in1=rb[:], op=OP.add)
            w0u = sp.tile([gp, 1], U32, tag=f"w0u{tagp}")
            nc.vector.tensor_copy(out=w0u[:], in_=w0[:])
            win = sp.tile([gp, W], F32, tag=f"win{tagp}")
            nc.gpsimd.indirect_dma_start(
                out=win[:],
                out_offset=None,
                in_=xh[:, None],
                in_offset=bass.IndirectOffsetOnAxis(ap=w0u[:, 0:1], axis=0),
            )
            sv = sp.tile([gp, 1], F32, tag=f"sv{tagp}")
            nc.vector.tensor_copy(out=sv[:], in_=ms[:, 0:1])
            st.update(scores=scores, sv=sv, w0=w0, win=win, m8=m8)
            return st

        def chain_p2(st, gp, tagp):
            w0, win, m8 = st["w0"], st["win"], st["m8"]
            mi = sp.tile([gp, 8], U32, tag=f"mi{tagp}")
            nc.vector.max_index(mi[:], m8[:], win[:])
            ii = sp.tile([gp, 1], F32, tag=f"ii{tagp}")
            nc.vector.tensor_copy(out=ii[:], in_=mi[:, 0:1])

            # neighborhood gather: start = peak - W = w0 + ii - W
            w2 = sp.tile([gp, 1], F32, tag=f"w2{tagp}")
            nc.vector.tensor_tensor(out=w2[:], in0=w0[:], in1=ii[:], op=OP.add)
            # shift + clamp + u32 cast in one casting tensor_scalar
            w2u = sp.tile([gp, 1], U32, tag=f"w2u{tagp}")
            nc.vector.tensor_scalar(
                out=w2u[:], in0=w2[:], scalar1=-float(W),
                scalar2=float(c.NPAD - c.NBW), op0=OP.add, op1=OP.min,
            )
            nb = sp.tile([gp, c.NBW], F32, tag=f"nb{tagp}")
            nc.gpsimd.indirect_dma_start(
                out=nb[:],
                out_offset=None,
                in_=xh[:, None],
                in_offset=bass.IndirectOffsetOnAxis(ap=w2u[:, 0:1], axis=0),
            )
            st.update(ii=ii, nb=nb)

        def chain_p3(st, gp, tagp):
            scores, sv, ii, nb = st["scores"], st["sv"], st["ii"], st["nb"]
            # px = ii, py = s, masked by score > 0
            O = sp.tile([gp, 3], F32, tag=f"O{tagp}")
            mkp = sp.tile([gp, 1], F32, tag=f"mkp{tagp}")
            nc.vector.tensor_scalar(
                out=mkp[:], in0=scores[:], scalar1=0.0, scalar2=None,
                op0=OP.is_gt,
            )
            nc.vector.tensor_tensor(
                out=O[:, 0:1], in0=ii[:], in1=mkp[:], op=OP.mult
            )
            nc.vector.tensor_tensor(
                out=O[:, 1:2], in0=sv[:], in1=mkp[:], op=OP.mult
            )  # sv is the winning segment index = py
            # interior = (0 < px < W-1) & (0 < py < H-1)
            ilo = sp.tile([gp, 2], F32, tag=f"ilo{tagp}")
            nc.vector.tensor_scalar(
                out=ilo[:], in0=O[:, 0:2], scalar1=0.0, scalar2=None,
                op0=OP.is_gt,
            )
            ihi = sp.tile([gp, 2], F32, tag=f"ihi{tagp}")
            nc.vector.tensor_tensor(
                out=ihi[:], in0=O[:, 0:2], in1=hi2[0:gp], op=OP.is_lt
            )
            nc.vector.tensor_tensor(out=ilo[:], in0=ilo[:], in1=ihi[:], op=OP.mult)
            intr = sp.tile([gp, 1], F32, tag=f"in{tagp}")
            nc.vector.tensor_reduce(out=intr[:], in_=ilo[:], axis=AX.X, op=OP.min)

            # dx = sign(nb[W+1] - nb[W-1]); dy = sign(nb[2W] - nb[0])
            D = sp.tile([gp, 2], F32, tag=f"D{tagp}")
            DL = sp.tile([gp, 2], F32, tag=f"DL{tagp}")
            for a, (ir, il) in enumerate(((W + 1, W - 1), (2 * W, 0))):
                nc.vector.tensor_tensor(
                    out=D[:, a : a + 1], in0=nb[:, ir : ir + 1],
                    in1=nb[:, il : il + 1], op=OP.is_gt,
                )
                nc.vector.tensor_tensor(
                    out=DL[:, a : a + 1], in0=nb[:, ir : ir + 1],
                    in1=nb[:, il : il + 1], op=OP.is_lt,
                )
            nc.vector.tensor_tensor(out=D[:], in0=D[:], in1=DL[:], op=OP.subtract)
            nc.vector.tensor_scalar(
                out=D[:], in0=D[:], scalar1=0.25, scalar2=None, op0=OP.mult
            )
            nc.vector.tensor_tensor(
                out=D[:], in0=D[:], in1=intr[:].to_broadcast([gp, 2]), op=OP.mult
            )
            nc.vector.tensor_tensor(out=O[:, 0:2], in0=O[:, 0:2], in1=D[:], op=OP.add)
            nc.vector.tensor_copy(out=O[:, 2:3], in_=scores[:])
            return O

        # ---- emission schedule ------------------------------------------
        # The stream phase is pure chunk reduces on DVE -- nothing else is
        # allowed to delay them, or ring-buffer reuse gates the chunk DMA
        # issues and the whole stream lock-steps. All phase-2 work happens
        # after the last chunk, with the rump chain interleaved into the
        # main chain's DMA-wait gaps (each chain's indirect-gather latency
        # is covered by the other chain's compute).
        for i in range(len(CHUNKS)):
            reduce_chunk(i)
        st_g = chain_p1(M[:], rb_g, c.P, "g")  # argseg + win_g gather
        nc.vector.reduce_max(
            out=Mr[:], in_=xtr[:].rearrange("p (s u) -> p s u", u=W), axis=AX.X
        )
        nc.gpsimd.dma_start(out=R8[:], in_=Mr[:])  # linearizing relayout
        st_r = chain_p1(R8[:], rb_r, c.RUMP, "r")
        chain_p2(st_g, c.P, "g")  # max_index + nb_g gather
        chain_p2(st_r, c.RUMP, "r")
        Og = chain_p3(st_g, c.P, "g")
        nc.sync.dma_start(out=oh[0 : c.P], in_=Og[:], single_packet=True)
        Or = chain_p3(st_r, c.RUMP, "r")
        nc.scalar.dma_start(out=oh[c.P : c.R], in_=Or[:], single_packet=True)

    nc.compile()
    return nc


def shard_inputs(cfg: Cfg, x: np.ndarray):
    c = cfg
    in_maps = []
    for k in range(c.ncores):
        shard = np.ascontiguousarray(
            x[k * c.BP : (k + 1) * c.BP], dtype=np.float32
        ).reshape(-1)
        xp = np.zeros(c.NPAD, np.float32)
        xp[c.FRONT : c.FRONT + c.SHN] = shard
        in_maps.append({"x": xp})
    return in_maps


def assemble_out(cfg: Cfg, per_core_outs):
    c = cfg
    outs = [o.reshape(c.BP, c.C, 3).astype(np.float32) for o in per_core_outs]
    return np.concatenate(outs, axis=0)


_PROGRAM = None


def _program():
    global _PROGRAM
    if _PROGRAM is None:
        _PROGRAM = build_program(CFG)
    return _PROGRAM


def kernel(x: np.ndarray) -> np.ndarray:
    from concourse.bass_utils import run_bass_kernel_spmd

    c = CFG
    assert x.shape == (c.B, c.C, c.H, c.W), x.shape
    nc = _program()
    in_maps = shard_inputs(c, np.asarray(x))
    res = run_bass_kernel_spmd(nc, in_maps, core_ids=list(range(c.ncores)))
    return assemble_out(c, [res.results[k]["out"] for k in range(c.ncores)])



# revision 2
# speedup vs baseline: 1.0369x; 1.0369x over previous
"""Trainium2 Bass kernel for HeatmapMaxDetBlock (argmax + local refinement).

Computes, for x[B, C, H, W]:
    scores = max over (H*W); idx = argmax; px = idx % W, py = idx // W (masked
    by score > 0); quarter-pixel refinement by sign of neighbor differences.
Returns [B, C, 3] = (px, py, scores).

Strategy (pure data parallel over 8 NeuronCores, batch-sharded; 136 heatmap
rows of H*W=49152 f32 per core):
  Rows live DIRECTLY on SBUF partitions. Main group = rows 0..127 on 128
  partitions; the free dim streams the 49152 row columns in 18 chunks
  (HWDGE on sync/scalar, 13-deep ring so DMA never gates on DVE).
  One DVE reduce per chunk produces per-(row, PAIR-of-lines) maxima with
  segment width 2*W=384 -- halving the per-segment DVE pipeline overhead
  vs one-line segments, which is what lets the reduces hide entirely
  under the ~70us DMA stream. The winning pair + in-pair argmax recover
  (py, px) with exact flat-argmax tie order.
  The 8 leftover rows (128..135) stream early via SWDGE as [128, 3072]
  (16 partitions per row); their ENTIRE detection chain (reduce,
  relayout, argmax, two small indirect gathers, refinement) runs
  mid-stream in DVE/gpsimd slack, so it adds nothing to the tail.
  The main group's tail is a single 4-line window gather (768 f32/row)
  followed by dense on-chip math: in-window argmax via max_index, then
  one fused scalar_tensor_tensor per direction (one-hot(iota==idx) *
  shifted-diff, accumulated) extracts the neighbor differences without a
  second dependent gather.
"""

import sys
from contextlib import ExitStack
from dataclasses import dataclass

import numpy as np

for _p in ("/opt/trn_rl_repo",):
    if _p not in sys.path:
        sys.path.insert(0, _p)

import concourse.bass as bass  # noqa: E402
import concourse.tile as tile  # noqa: E402
from concourse import bacc, mybir  # noqa: E402

F32 = mybir.dt.float32
I32 = mybir.dt.int32
U32 = mybir.dt.uint32
AX = mybir.AxisListType
OP = mybir.AluOpType


@dataclass(frozen=True)
class Cfg:
    B: int = 64
    C: int = 17
    H: int = 256
    W: int = 192
    ncores: int = 8
    P: int = 128
    FRONT: int = 256
    REAR: int = 512

    @property
    def BP(self):  # batches per core
        return self.B // self.ncores

    @property
    def R(self):  # heatmap rows per core
        return self.BP * self.C

    @property
    def HWm(self):
        return self.H * self.W

    @property
    def SEGW(self):  # segment = PAIR of heatmap lines
        return 2 * self.W

    @property
    def NSEG(self):  # pair-segments per row
        return self.H // 2

    @property
    def WINW(self):  # gathered window: pair + one line each side = 4 lines
        return 4 * self.W

    @property
    def SHN(self):
        return self.R * self.HWm

    @property
    def NPAD(self):
        return self.FRONT + self.SHN + self.REAR

    @property
    def RUMP(self):  # leftover rows beyond the 128-partition main group
        return self.R - self.P


CFG = Cfg()

# Column chunking of the main group's stream. Two small chunks first so the
# earliest reduces (which later DMAs' semaphore-lane reuse gates on) finish
# early; small chunks at the end so the final reduce -> argmax chain starts
# as soon as possible after the last byte lands. Chunks alternate between
# the two HWDGE queues.
CHUNKS = [1536, 1536] + [3072] * 14 + [2304, 768]
assert sum(CHUNKS) == CFG.HWm and all(c % CFG.SEGW == 0 for c in CHUNKS)
NBUFS = 13  # ring depth; 13 x 3072 x 4B = 156 KiB/partition


def build_program(cfg: Cfg):
    c = cfg
    W = c.W
    SW = c.SEGW
    assert c.RUMP * 16 == c.P, "rump layout assumes 16 partitions per row"
    assert c.FRONT >= W and c.REAR >= 2 * W

    nc = bacc.Bacc(
        "TRN2", target_bir_lowering=False, debug=False, num_devices=c.ncores
    )
    xh = nc.dram_tensor("x", [c.NPAD], F32, kind="ExternalInput").ap()
    oh = nc.dram_tensor("out", [c.R, 3], F32, kind="ExternalOutput").ap()

    with ExitStack() as ctx:
        tc = ctx.enter_context(tile.TileContext(nc))
        xpool = ctx.enter_context(tc.tile_pool(name="xp", bufs=NBUFS))
        sp = ctx.enter_context(tc.tile_pool(name="sp", bufs=1))

        # ---- stream DMAs (all issued up front) ---------------------------
        xts = []
        col = 0
        for i, ncols in enumerate(CHUNKS):
            xt = xpool.tile([c.P, max(CHUNKS)], F32, tag="xt")
            src = bass.AP(
                xh.tensor, c.FRONT + col, [[c.HWm, c.P], [1, ncols]]
            )
            eng = nc.sync if i % 2 == 0 else nc.scalar
            eng.dma_start(out=xt[:, 0:ncols], in_=src)
            xts.append((xt, col, ncols))
            col += ncols

        # rump rows ride the (otherwise idle) SWDGE queue; they land early
        # and their whole detection chain hides under the main stream
        xtr = sp.tile([c.P, 3072], F32, tag="xtr")
        rsrc = bass.AP(
            xh.tensor,
            c.FRONT + c.P * c.HWm,
            [[c.HWm, c.RUMP], [3072, 16], [1, 3072]],
        )
        nc.gpsimd.dma_start(out=xtr[:], in_=rsrc)

        # ---- on-chip constants (cheap; off the critical path) ------------
        # values stay below 2^24 so f32 iota is exact
        rb_g = sp.tile([c.P, 1], F32, tag="rb_g")
        nc.gpsimd.iota(rb_g[:], pattern=[[0, 1]], base=c.FRONT,
                       channel_multiplier=c.HWm,
                       allow_small_or_imprecise_dtypes=True)
        rb_r = sp.tile([c.RUMP, 1], F32, tag="rb_r")
        nc.gpsimd.iota(rb_r[:], pattern=[[0, 1]],
                       base=c.FRONT + c.P * c.HWm, channel_multiplier=c.HWm,
                       allow_small_or_imprecise_dtypes=True)
        iot = sp.tile([c.P, SW], F32, tag="iot")  # 0..383 per partition
        nc.gpsimd.iota(iot[:], pattern=[[1, SW]], base=0,
                       channel_multiplier=0,
                       allow_small_or_imprecise_dtypes=True)
        # interior upper bounds (px < W-1, py < H-1)
        hi2 = sp.tile([c.P, 2], F32, tag="hi2")
        nc.vector.memset(hi2[:, 0:1], float(W - 1))
        nc.vector.memset(hi2[:, 1:2], float(c.H - 1))

        # ---- phase 1 reduces --------------------------------------------
        Mr = sp.tile([c.P, 8], F32, tag="Mr")
        M = sp.tile([c.P, c.NSEG], F32, tag="M")

        def reduce_chunk(i):
            xt, col, ncols = xts[i]
            s0 = col // SW
            ns = ncols // SW
            nc.vector.reduce_max(
                out=M[:, s0 : s0 + ns],
                in_=xt[:, 0:ncols].rearrange("p (s u) -> p s u", u=SW),
                axis=AX.X,
            )

        # rump pair-maxima relayout target: [8 rows, 128 pair-segments]
        R8 = sp.tile([c.RUMP, c.NSEG], F32, tag="R8")

        # ---- emission schedule ------------------------------------------
        # DVE order: chunk reduces with the rump chain's (cheap) ops
        # interleaved mid-stream; the ring depth absorbs the transient DVE
        # lag. All main-group tail work comes after the last chunk reduce.
        for i in range(6):
            reduce_chunk(i)

        # rump reduce + relayout (data landed long ago)
        nc.vector.reduce_max(
            out=Mr[:], in_=xtr[:].rearrange("p (s u) -> p s u", u=SW), axis=AX.X
        )
        nc.gpsimd.dma_start(out=R8[:], in_=Mr[:])  # linearizing relayout

        for i in range(6, 9):
            reduce_chunk(i)

        # rump p1: scores + winning pair + pair-window gather
        sc_r = sp.tile([c.RUMP, 1], F32, tag="sc_r")
        nc.vector.reduce_max(out=sc_r[:], in_=R8[:], axis=AX.X)
        m8r = sp.tile([c.RUMP, 8], F32, tag="m8r")
        nc.vector.tensor_copy(out=m8r[:], in_=sc_r[:].to_broadcast([c.RUMP, 8]))
        msr = sp.tile([c.RUMP, 8], U32, tag="msr")
        nc.vector.max_index(msr[:], m8r[:], R8[:])
        sv_r = sp.tile([c.RUMP, 1], F32, tag="sv_r")
        nc.vector.tensor_copy(out=sv_r[:], in_=msr[:, 0:1])
        w0r = sp.tile([c.RUMP, 1], F32, tag="w0r")
        nc.vector.scalar_tensor_tensor(
            out=w0r[:], in0=sv_r[:], scalar=float(SW), in1=rb_r[:],
            op0=OP.mult, op1=OP.add,
        )
        w0ru = sp.tile([c.RUMP, 1], U32, tag="w0ru")
        nc.vector.tensor_copy(out=w0ru[:], in_=w0r[:])
        win_r = sp.tile([c.RUMP, SW], F32, tag="win_r")
        nc.gpsimd.indirect_dma_start(
            out=win_r[:],
            out_offset=None,
            in_=xh[:, None],
            in_offset=bass.IndirectOffsetOnAxis(ap=w0ru[:, 0:1], axis=0),
        )

        for i in range(9, 11):
            reduce_chunk(i)

        # rump p2: in-pair argmax -> neighborhood gather
        mir = sp.tile([c.RUMP, 8], U32, tag="mir")
        nc.vector.max_index(mir[:], m8r[:], win_r[:])
        fir = sp.tile([c.RUMP, 1], F32, tag="fir")
        nc.vector.tensor_copy(out=fir[:], in_=mir[:, 0:1])
        w2r = sp.tile([c.RUMP, 1], F32, tag="w2r")
        nc.vector.scalar_tensor_tensor(
            out=w2r[:], in0=fir[:], scalar=-float(W), in1=w0r[:],
            op0=OP.add, op1=OP.add,
        )
        w2ru = sp.tile([c.RUMP, 1], U32, tag="w2ru")
        nc.vector.tensor_copy(out=w2ru[:], in_=w2r[:])
        nb_r = sp.tile([c.RUMP, 2 * W + 1], F32, tag="nb_r")
        nc.gpsimd.indirect_dma_start(
            out=nb_r[:],
            out_offset=None,
            in_=xh[:, None],
            in_offset=bass.IndirectOffsetOnAxis(ap=w2ru[:, 0:1], axis=0),
        )

        for i in range(11, 13):
            reduce_chunk(i)

        # rump p3: px/py assembly + quarter-pixel refinement (tiny ops)
        gp = c.RUMP
        liner = sp.tile([gp, 1], F32, tag="liner")
        nc.vector.tensor_scalar(
            out=liner[:], in0=fir[:], scalar1=float(W), scalar2=None,
            op0=OP.is_ge,
        )
        Or = sp.tile([gp, 3], F32, tag="Or")
        pxr = sp.tile([gp, 1], F32, tag="pxr")
        nc.vector.scalar_tensor_tensor(
            out=pxr[:], in0=liner[:], scalar=-float(W), in1=fir[:],
            op0=OP.mult, op1=OP.add,
        )
        pyr = sp.tile([gp, 1], F32, tag="pyr")
        nc.vector.scalar_tensor_tensor(
            out=pyr[:], in0=sv_r[:], scalar=2.0, in1=liner[:],
            op0=OP.mult, op1=OP.add,
        )
        mkpr = sp.tile([gp, 1], F32, tag="mkpr")
        nc.vector.tensor_scalar(
            out=mkpr[:], in0=sc_r[:], scalar1=0.0, scalar2=None, op0=OP.is_gt,
        )
        nc.vector.tensor_tensor(out=Or[:, 0:1], in0=pxr[:], in1=mkpr[:], op=OP.mult)
        nc.vector.tensor_tensor(out=Or[:, 1:2], in0=pyr[:], in1=mkpr[:], op=OP.mult)
        ilor = sp.tile([gp, 2], F32, tag="ilor")
        nc.vector.tensor_scalar(
            out=ilor[:], in0=Or[:, 0:2], scalar1=0.0, scalar2=None, op0=OP.is_gt,
        )
        ihir = sp.tile([gp, 2], F32, tag="ihir")
        nc.vector.tensor_tensor(out=ihir[:], in0=Or[:, 0:2], in1=hi2[0:gp], op=OP.is_lt)
        nc.vector.tensor_tensor(out=ilor[:], in0=ilor[:], in1=ihir[:], op=OP.mult)
        intr_r = sp.tile([gp, 1], F32, tag="intr_r")
        nc.vector.tensor_reduce(out=intr_r[:], in_=ilor[:], axis=AX.X, op=OP.min)
        Dr = sp.tile([gp, 2], F32, tag="Dr")
        DLr = sp.tile([gp, 2], F32, tag="DLr")
        for a, (ir, il) in enumerate(((W + 1, W - 1), (2 * W, 0))):
            nc.vector.tensor_tensor(
                out=Dr[:, a : a + 1], in0=nb_r[:, ir : ir + 1],
                in1=nb_r[:, il : il + 1], op=OP.is_gt,
            )
            nc.vector.tensor_tensor(
                out=DLr[:, a : a + 1], in0=nb_r[:, ir : ir + 1],
                in1=nb_r[:, il : il + 1], op=OP.is_lt,
            )
        nc.vector.tensor_tensor(out=Dr[:], in0=Dr[:], in1=DLr[:], op=OP.subtract)
        nc.vector.tensor_scalar(
            out=Dr[:], in0=Dr[:], scalar1=0.25, scalar2=None, op0=OP.mult
        )
        nc.vector.tensor_tensor(
            out=Dr[:], in0=Dr[:], in1=intr_r[:].to_broadcast([gp, 2]), op=OP.mult
        )
        nc.vector.tensor_tensor(out=Or[:, 0:2], in0=Or[:, 0:2], in1=Dr[:], op=OP.add)
        nc.vector.tensor_copy(out=Or[:, 2:3], in_=sc_r[:])
        # rump output leaves mid-stream (emitted after all scalar chunk DMAs)
        nc.scalar.dma_start(out=oh[c.P : c.R], in_=Or[:], single_packet=True)

        for i in range(13, len(CHUNKS)):
            reduce_chunk(i)

        # ---- main-group tail --------------------------------------------
        gp = c.P
        scores = sp.tile([gp, 1], F32, tag="sc_g")
        nc.vector.reduce_max(out=scores[:], in_=M[:], axis=AX.X)
        m8 = sp.tile([gp, 8], F32, tag="m8g")
        nc.vector.tensor_copy(out=m8[:], in_=scores[:].to_broadcast([gp, 8]))
        ms = sp.tile([gp, 8], U32, tag="msg")
        nc.vector.max_index(ms[:], m8[:], M[:])
        # window start (absolute in padded x): w0 = rb + SW*s - W
        sv = sp.tile([gp, 1], F32, tag="sv_g")
        nc.vector.tensor_copy(out=sv[:], in_=ms[:, 0:1])
        w0a = sp.tile([gp, 1], F32, tag="w0a")
        nc.vector.tensor_scalar(
            out=w0a[:], in0=ms[:, 0:1], scalar1=float(SW), scalar2=-float(W),
            op0=OP.mult, op1=OP.add,
        )
        w0 = sp.tile([gp, 1], F32, tag="w0g")
        nc.vector.tensor_tensor(out=w0[:], in0=w0a[:], in1=rb_g[:], op=OP.add)
        w0u = sp.tile([gp, 1], U32, tag="w0ug")
        nc.vector.tensor_copy(out=w0u[:], in_=w0[:])
        win = sp.tile([gp, c.WINW], F32, tag="win_g")
        nc.gpsimd.indirect_dma_start(
            out=win[:],
            out_offset=None,
            in_=xh[:, None],
            in_offset=bass.IndirectOffsetOnAxis(ap=w0u[:, 0:1], axis=0),
        )
        # in-pair argmax over the window's middle 384 columns
        mi = sp.tile([gp, 8], U32, tag="mig")
        nc.vector.max_index(mi[:], m8[:], win[:, W : W + SW])
        fi = sp.tile([gp, 1], F32, tag="fig")
        nc.vector.tensor_copy(out=fi[:], in_=mi[:, 0:1])
        # px/py assembly (before refinement, as the reference does)
        line = sp.tile([gp, 1], F32, tag="lineg")
        nc.vector.tensor_scalar(
            out=line[:], in0=fi[:], scalar1=float(W), scalar2=None, op0=OP.is_ge,
        )
        px = sp.tile([gp, 1], F32, tag="pxg")
        nc.vector.scalar_tensor_tensor(
            out=px[:], in0=line[:], scalar=-float(W), in1=fi[:],
            op0=OP.mult, op1=OP.add,
        )
        py = sp.tile([gp, 1], F32, tag="pyg")
        nc.vector.scalar_tensor_tensor(
            out=py[:], in0=sv[:], scalar=2.0, in1=line[:],
            op0=OP.mult, op1=OP.add,
        )
        mkp = sp.tile([gp, 1], F32, tag="mkpg")
        nc.vector.tensor_scalar(
            out=mkp[:], in0=scores[:], scalar1=0.0, scalar2=None, op0=OP.is_gt,
        )
        O = sp.tile([gp, 3], F32, tag="Og")
        nc.vector.tensor_tensor(out=O[:, 0:1], in0=px[:], in1=mkp[:], op=OP.mult)
        nc.vector.tensor_tensor(out=O[:, 1:2], in0=py[:], in1=mkp[:], op=OP.mult)
        ilo = sp.tile([gp, 2], F32, tag="ilog")
        nc.vector.tensor_scalar(
            out=ilo[:], in0=O[:, 0:2], scalar1=0.0, scalar2=None, op0=OP.is_gt,
        )
        ihi = sp.tile([gp, 2], F32, tag="ihig")
        nc.vector.tensor_tensor(out=ihi[:], in0=O[:, 0:2], in1=hi2[0:gp], op=OP.is_lt)
        nc.vector.tensor_tensor(out=ilo[:], in0=ilo[:], in1=ihi[:], op=OP.mult)
        intr = sp.tile([gp, 1], F32, tag="intr_g")
        nc.vector.tensor_reduce(out=intr[:], in_=ilo[:], axis=AX.X, op=OP.min)
        # shifted differences over the pair region, then one fused
        # one-hot-select+accumulate per direction
        DH = sp.tile([gp, SW], F32, tag="DHg")
        nc.vector.tensor_tensor(
            out=DH[:], in0=win[:, W + 1 : W + SW + 1],
            in1=win[:, W - 1 : W + SW - 1], op=OP.subtract,
        )
        DV = sp.tile([gp, SW], F32, tag="DVg")
        nc.vector.tensor_tensor(
            out=DV[:], in0=win[:, SW : SW + SW], in1=win[:, 0:SW], op=OP.subtract,
        )
        D2 = sp.tile([gp, 2], F32, tag="D2g")
        junkH = sp.tile([gp, SW], F32, tag="junkH")
        nc.vector.scalar_tensor_tensor(
            out=junkH[:], in0=iot[:], scalar=fi[:, 0:1], in1=DH[:],
            op0=OP.is_equal, op1=OP.mult, accum_out=D2[:, 0:1],
        )
        junkV = sp.tile([gp, SW], F32, tag="junkV")
        nc.vector.scalar_tensor_tensor(
            out=junkV[:], in0=iot[:], scalar=fi[:, 0:1], in1=DV[:],
            op0=OP.is_equal, op1=OP.mult, accum_out=D2[:, 1:2],
        )
        # D = sign(D2) * 0.25 * interior
        Dg = sp.tile([gp, 2], F32, tag="Dg")
        DLg = sp.tile([gp, 2], F32, tag="DLg")
        nc.vector.tensor_scalar(
            out=Dg[:], in0=D2[:], scalar1=0.0, scalar2=None, op0=OP.is_gt,
        )
        nc.vector.tensor_scalar(
            out=DLg[:], in0=D2[:], scalar1=0.0, scalar2=None, op0=OP.is_lt,
        )
        nc.vector.tensor_tensor(out=Dg[:], in0=Dg[:], in1=DLg[:], op=OP.subtract)
        nc.vector.tensor_scalar(
            out=Dg[:], in0=Dg[:], scalar1=0.25, scalar2=None, op0=OP.mult
        )
        nc.vector.tensor_tensor(
            out=Dg[:], in0=Dg[:], in1=intr[:].to_broadcast([gp, 2]), op=OP.mult
        )
        nc.vector.tensor_tensor(out=O[:, 0:2], in0=O[:, 0:2], in1=Dg[:], op=OP.add)
        nc.vector.tensor_copy(out=O[:, 2:3], in_=scores[:])
        nc.sync.dma_start(out=oh[0 : c.P], in_=O[:], single_packet=True)

    nc.compile()
    return nc


def shard_inputs(cfg: Cfg, x: np.ndarray):
    c = cfg
    in_maps = []
    for k in range(c.ncores):
        shard = np.ascontiguousarray(
            x[k * c.BP : (k + 1) * c.BP], dtype=np.float32
        ).reshape(-1)
        xp = np.zeros(c.NPAD, np.float32)
        xp[c.FRONT : c.FRONT + c.SHN] = shard
        in_maps.append({"x": xp})
    return in_maps


def assemble_out(cfg: Cfg, per_core_outs):
    c = cfg
    outs = [o.reshape(c.BP, c.C, 3).astype(np.float32) for o in per_core_outs]
    return np.concatenate(outs, axis=0)


_PROGRAM = None


def _program():
    global _PROGRAM
    if _PROGRAM is None:
        _PROGRAM = build_program(CFG)
    return _PROGRAM


def kernel(x: np.ndarray) -> np.ndarray:
    from concourse.bass_utils import run_bass_kernel_spmd

    c = CFG
    assert x.shape == (c.B, c.C, c.H, c.W), x.shape
    nc = _program()
    in_maps = shard_inputs(c, np.asarray(x))
    res = run_bass_kernel_spmd(nc, in_maps, core_ids=list(range(c.ncores)))
    return assemble_out(c, [res.results[k]["out"] for k in range(c.ncores)])


# revision 7
# speedup vs baseline: 1.0639x; 1.0260x over previous
"""Trainium2 Bass kernel for HeatmapMaxDetBlock (argmax + local refinement).

Computes, for x[B, C, H, W]:
    scores = max over (H*W); idx = argmax; px = idx % W, py = idx // W (masked
    by score > 0); quarter-pixel refinement by sign of neighbor differences.
Returns [B, C, 3] = (px, py, scores).

Strategy (pure data parallel over 8 NeuronCores, batch-sharded; 136 heatmap
rows of H*W=49152 f32 per core):
  Rows live DIRECTLY on SBUF partitions. Main group = rows 0..127 on 128
  partitions; the free dim streams the 49152 row columns in 18 chunks
  (HWDGE on sync/scalar, 13-deep ring so DMA never gates on DVE).
  One DVE reduce per chunk produces per-(row, PAIR-of-lines) maxima with
  segment width 2*W=384 -- halving the per-segment DVE pipeline overhead
  vs one-line segments, which is what lets the reduces hide entirely
  under the ~70us DMA stream. The winning pair + in-pair argmax recover
  (py, px) with exact flat-argmax tie order.
  The 8 leftover rows (128..135) stream early via SWDGE as [128, 3072]
  (16 partitions per row); their ENTIRE detection chain (reduce,
  relayout, argmax, two small indirect gathers, refinement) runs
  mid-stream in DVE/gpsimd slack, so it adds nothing to the tail.
  The main group's tail is a single 4-line window gather (768 f32/row)
  followed by dense on-chip math: in-window argmax via max_index, then
  one fused scalar_tensor_tensor per direction (one-hot(iota==idx) *
  shifted-diff, accumulated) extracts the neighbor differences without a
  second dependent gather.
"""

import sys
from contextlib import ExitStack
from dataclasses import dataclass

import numpy as np

for _p in ("/opt/trn_rl_repo",):
    if _p not in sys.path:
        sys.path.insert(0, _p)

import concourse.bass as bass  # noqa: E402
import concourse.tile as tile  # noqa: E402
from concourse import bacc, mybir  # noqa: E402

F32 = mybir.dt.float32
I32 = mybir.dt.int32
U32 = mybir.dt.uint32
AX = mybir.AxisListType
OP = mybir.AluOpType


@dataclass(frozen=True)
class Cfg:
    B: int = 64
    C: int = 17
    H: int = 256
    W: int = 192
    ncores: int = 8
    P: int = 128
    FRONT: int = 256
    REAR: int = 512

    @property
    def BP(self):  # batches per core
        return self.B // self.ncores

    @property
    def R(self):  # heatmap rows per core
        return self.BP * self.C

    @property
    def HWm(self):
        return self.H * self.W

    @property
    def SEGW(self):  # segment = PAIR of heatmap lines
        return 2 * self.W

    @property
    def NSEG(self):  # pair-segments per row
        return self.H // 2

    @property
    def WINW(self):  # gathered window: pair + one line each side = 4 lines
        return 4 * self.W

    @property
    def SHN(self):
        return self.R * self.HWm

    @property
    def NPAD(self):
        return self.FRONT + self.SHN + self.REAR

    @property
    def RUMP(self):  # leftover rows beyond the 128-partition main group
        return self.R - self.P


CFG = Cfg()

# Column chunking of the main group's stream. There are 8 HWDGE semaphore
# lanes shared by the two HWDGE queues: the (i+8)-th HWDGE DMA can only
# issue after the reduce consuming lane-mate DMA i ran. The ramp of small
# chunks up front makes those early reduces finish fast, so the big
# steady-state chunks stream without ever waiting on DVE progress; small
# chunks at the end let the final reduce -> argmax chain start as soon as
# possible after the last byte lands.
CHUNKS = [768, 768, 1536, 1536, 3072, 3072, 3072, 3072, 3072,
          4224, 4224, 4224, 4224, 4224, 3840, 2688, 768, 768]
assert sum(CHUNKS) == CFG.HWm and all(c % CFG.SEGW == 0 for c in CHUNKS)
NBUFS = 9  # ring depth; 9 x 4224 x 4B = 148.5 KiB/partition


def build_program(cfg: Cfg):
    c = cfg
    W = c.W
    SW = c.SEGW
    assert c.RUMP * 16 == c.P, "rump layout assumes 16 partitions per row"
    assert c.FRONT >= W and c.REAR >= 2 * W

    nc = bacc.Bacc(
        "TRN2", target_bir_lowering=False, debug=False, num_devices=c.ncores
    )
    xh = nc.dram_tensor("x", [c.NPAD], F32, kind="ExternalInput").ap()
    oh = nc.dram_tensor("out", [c.R, 3], F32, kind="ExternalOutput").ap()

    with ExitStack() as ctx:
        tc = ctx.enter_context(tile.TileContext(nc))
        xpool = ctx.enter_context(tc.tile_pool(name="xp", bufs=NBUFS))
        sp = ctx.enter_context(tc.tile_pool(name="sp", bufs=1))

        # ---- stream DMAs (all issued up front) ---------------------------
        xts = []
        col = 0
        for i, ncols in enumerate(CHUNKS):
            xt = xpool.tile([c.P, max(CHUNKS)], F32, tag="xt")
            src = bass.AP(
                xh.tensor, c.FRONT + col, [[c.HWm, c.P], [1, ncols]]
            )
            eng = nc.sync if i % 2 == 0 else nc.scalar
            eng.dma_start(out=xt[:, 0:ncols], in_=src)
            xts.append((xt, col, ncols))
            col += ncols

        # rump rows ride the (otherwise idle) SWDGE queue; they land early
        # and their whole detection chain hides under the main stream
        xtr = sp.tile([c.P, 3072], F32, tag="xtr")
        rsrc = bass.AP(
            xh.tensor,
            c.FRONT + c.P * c.HWm,
            [[c.HWm, c.RUMP], [3072, 16], [1, 3072]],
        )
        nc.gpsimd.dma_start(out=xtr[:], in_=rsrc)

        # ---- on-chip constants (cheap; off the critical path) ------------
        # values stay below 2^24 so f32 iota is exact
        rb_g = sp.tile([c.P, 1], F32, tag="rb_g")
        nc.gpsimd.iota(rb_g[:], pattern=[[0, 1]], base=c.FRONT,
                       channel_multiplier=c.HWm,
                       allow_small_or_imprecise_dtypes=True)
        rb_r = sp.tile([c.RUMP, 1], F32, tag="rb_r")
        nc.gpsimd.iota(rb_r[:], pattern=[[0, 1]],
                       base=c.FRONT + c.P * c.HWm, channel_multiplier=c.HWm,
                       allow_small_or_imprecise_dtypes=True)
        iot = sp.tile([c.P, SW], F32, tag="iot")  # 0..383 per partition
        nc.gpsimd.iota(iot[:], pattern=[[1, SW]], base=0,
                       channel_multiplier=0,
                       allow_small_or_imprecise_dtypes=True)
        # interior upper bounds (px < W-1, py < H-1)
        hi2 = sp.tile([c.P, 2], F32, tag="hi2")
        nc.vector.memset(hi2[:, 0:1], float(W - 1))
        nc.vector.memset(hi2[:, 1:2], float(c.H - 1))

        # ---- phase 1 reduces --------------------------------------------
        Mr = sp.tile([c.P, 8], F32, tag="Mr")
        M = sp.tile([c.P, c.NSEG], F32, tag="M")

        def reduce_chunk(i):
            xt, col, ncols = xts[i]
            s0 = col // SW
            ns = ncols // SW
            nc.vector.reduce_max(
                out=M[:, s0 : s0 + ns],
                in_=xt[:, 0:ncols].rearrange("p (s u) -> p s u", u=SW),
                axis=AX.X,
            )

        # rump pair-maxima relayout target: [8 rows, 128 pair-segments]
        R8 = sp.tile([c.RUMP, c.NSEG], F32, tag="R8")

        # ---- emission schedule ------------------------------------------
        # DVE order: chunk reduces with the rump chain's (cheap) ops
        # interleaved mid-stream, placed in the early ramp where DVE has
        # slack. All main-group tail work comes after the last chunk reduce.
        for i in range(4):
            reduce_chunk(i)

        # rump reduce + relayout (rump data streams in early via SWDGE)
        nc.vector.reduce_max(
            out=Mr[:], in_=xtr[:].rearrange("p (s u) -> p s u", u=SW), axis=AX.X
        )
        nc.gpsimd.dma_start(out=R8[:], in_=Mr[:])  # linearizing relayout

        for i in range(4, 6):
            reduce_chunk(i)

        # rump p1: scores + winning pair + pair-window gather
        sc_r = sp.tile([c.RUMP, 1], F32, tag="sc_r")
        nc.vector.reduce_max(out=sc_r[:], in_=R8[:], axis=AX.X)
        m8r = sp.tile([c.RUMP, 8], F32, tag="m8r")
        nc.vector.tensor_copy(out=m8r[:], in_=sc_r[:].to_broadcast([c.RUMP, 8]))
        msr = sp.tile([c.RUMP, 8], U32, tag="msr")
        nc.vector.max_index(msr[:], m8r[:], R8[:])
        sv_r = sp.tile([c.RUMP, 1], F32, tag="sv_r")
        nc.vector.tensor_copy(out=sv_r[:], in_=msr[:, 0:1])
        w0r = sp.tile([c.RUMP, 1], F32, tag="w0r")
        nc.vector.scalar_tensor_tensor(
            out=w0r[:], in0=sv_r[:], scalar=float(SW), in1=rb_r[:],
            op0=OP.mult, op1=OP.add,
        )
        w0ru = sp.tile([c.RUMP, 1], U32, tag="w0ru")
        nc.vector.tensor_copy(out=w0ru[:], in_=w0r[:])
        win_r = sp.tile([c.RUMP, SW], F32, tag="win_r")
        nc.gpsimd.indirect_dma_start(
            out=win_r[:],
            out_offset=None,
            in_=xh[:, None],
            in_offset=bass.IndirectOffsetOnAxis(ap=w0ru[:, 0:1], axis=0),
        )

        for i in range(6, 8):
            reduce_chunk(i)

        # rump p2: in-pair argmax -> neighborhood gather
        mir = sp.tile([c.RUMP, 8], U32, tag="mir")
        nc.vector.max_index(mir[:], m8r[:], win_r[:])
        fir = sp.tile([c.RUMP, 1], F32, tag="fir")
        nc.vector.tensor_copy(out=fir[:], in_=mir[:, 0:1])
        w2r = sp.tile([c.RUMP, 1], F32, tag="w2r")
        nc.vector.scalar_tensor_tensor(
            out=w2r[:], in0=fir[:], scalar=-float(W), in1=w0r[:],
            op0=OP.add, op1=OP.add,
        )
        w2ru = sp.tile([c.RUMP, 1], U32, tag="w2ru")
        nc.vector.tensor_copy(out=w2ru[:], in_=w2r[:])
        nb_r = sp.tile([c.RUMP, 2 * W + 1], F32, tag="nb_r")
        nc.gpsimd.indirect_dma_start(
            out=nb_r[:],
            out_offset=None,
            in_=xh[:, None],
            in_offset=bass.IndirectOffsetOnAxis(ap=w2ru[:, 0:1], axis=0),
        )

        for i in range(8, 10):
            reduce_chunk(i)

        # rump p3: px/py assembly + quarter-pixel refinement (tiny ops)
        gp = c.RUMP
        liner = sp.tile([gp, 1], F32, tag="liner")
        nc.vector.tensor_scalar(
            out=liner[:], in0=fir[:], scalar1=float(W), scalar2=None,
            op0=OP.is_ge,
        )
        Or = sp.tile([gp, 3], F32, tag="Or")
        pxr = sp.tile([gp, 1], F32, tag="pxr")
        nc.vector.scalar_tensor_tensor(
            out=pxr[:], in0=liner[:], scalar=-float(W), in1=fir[:],
            op0=OP.mult, op1=OP.add,
        )
        pyr = sp.tile([gp, 1], F32, tag="pyr")
        nc.vector.scalar_tensor_tensor(
            out=pyr[:], in0=sv_r[:], scalar=2.0, in1=liner[:],
            op0=OP.mult, op1=OP.add,
        )
        mkpr = sp.tile([gp, 1], F32, tag="mkpr")
        nc.vector.tensor_scalar(
            out=mkpr[:], in0=sc_r[:], scalar1=0.0, scalar2=None, op0=OP.is_gt,
        )
        nc.vector.tensor_tensor(out=Or[:, 0:1], in0=pxr[:], in1=mkpr[:], op=OP.mult)
        nc.vector.tensor_tensor(out=Or[:, 1:2], in0=pyr[:], in1=mkpr[:], op=OP.mult)
        ilor = sp.tile([gp, 2], F32, tag="ilor")
        nc.vector.tensor_scalar(
            out=ilor[:], in0=Or[:, 0:2], scalar1=0.0, scalar2=None, op0=OP.is_gt,
        )
        ihir = sp.tile([gp, 2], F32, tag="ihir")
        nc.vector.tensor_tensor(out=ihir[:], in0=Or[:, 0:2], in1=hi2[0:gp], op=OP.is_lt)
        nc.vector.tensor_tensor(out=ilor[:], in0=ilor[:], in1=ihir[:], op=OP.mult)
        intr_r = sp.tile([gp, 1], F32, tag="intr_r")
        nc.vector.tensor_reduce(out=intr_r[:], in_=ilor[:], axis=AX.X, op=OP.min)
        Dr = sp.tile([gp, 2], F32, tag="Dr")
        DLr = sp.tile([gp, 2], F32, tag="DLr")
        for a, (ir, il) in enumerate(((W + 1, W - 1), (2 * W, 0))):
            nc.vector.tensor_tensor(
                out=Dr[:, a : a + 1], in0=nb_r[:, ir : ir + 1],
                in1=nb_r[:, il : il + 1], op=OP.is_gt,
            )
            nc.vector.tensor_tensor(
                out=DLr[:, a : a + 1], in0=nb_r[:, ir : ir + 1],
                in1=nb_r[:, il : il + 1], op=OP.is_lt,
            )
        nc.vector.tensor_tensor(out=Dr[:], in0=Dr[:], in1=DLr[:], op=OP.subtract)
        nc.vector.tensor_scalar(
            out=Dr[:], in0=Dr[:], scalar1=0.25, scalar2=None, op0=OP.mult
        )
        nc.vector.tensor_tensor(
            out=Dr[:], in0=Dr[:], in1=intr_r[:].to_broadcast([gp, 2]), op=OP.mult
        )
        nc.vector.tensor_tensor(out=Or[:, 0:2], in0=Or[:, 0:2], in1=Dr[:], op=OP.add)
        nc.vector.tensor_copy(out=Or[:, 2:3], in_=sc_r[:])
        # rump output leaves mid-stream (emitted after all scalar chunk DMAs)
        nc.scalar.dma_start(out=oh[c.P : c.R], in_=Or[:], single_packet=True)

        for i in range(10, len(CHUNKS)):
            reduce_chunk(i)

        # ---- main-group tail --------------------------------------------
        gp = c.P
        scores = sp.tile([gp, 1], F32, tag="sc_g")
        nc.vector.reduce_max(out=scores[:], in_=M[:], axis=AX.X)
        m8 = sp.tile([gp, 8], F32, tag="m8g")
        nc.vector.tensor_copy(out=m8[:], in_=scores[:].to_broadcast([gp, 8]))
        ms = sp.tile([gp, 8], U32, tag="msg")
        nc.vector.max_index(ms[:], m8[:], M[:])
        # window start (absolute in padded x): w0 = rb + SW*s - W
        sv = sp.tile([gp, 1], F32, tag="sv_g")
        nc.vector.tensor_copy(out=sv[:], in_=ms[:, 0:1])
        w0a = sp.tile([gp, 1], F32, tag="w0a")
        nc.vector.tensor_scalar(
            out=w0a[:], in0=ms[:, 0:1], scalar1=float(SW), scalar2=-float(W),
            op0=OP.mult, op1=OP.add,
        )
        w0 = sp.tile([gp, 1], F32, tag="w0g")
        nc.vector.tensor_tensor(out=w0[:], in0=w0a[:], in1=rb_g[:], op=OP.add)
        w0u = sp.tile([gp, 1], U32, tag="w0ug")
        nc.vector.tensor_copy(out=w0u[:], in_=w0[:])
        win = sp.tile([gp, c.WINW], F32, tag="win_g")
        nc.gpsimd.indirect_dma_start(
            out=win[:],
            out_offset=None,
            in_=xh[:, None],
            in_offset=bass.IndirectOffsetOnAxis(ap=w0u[:, 0:1], axis=0),
        )
        # in-pair argmax over the window's middle 384 columns
        mi = sp.tile([gp, 8], U32, tag="mig")
        nc.vector.max_index(mi[:], m8[:], win[:, W : W + SW])
        fi = sp.tile([gp, 1], F32, tag="fig")
        nc.vector.tensor_copy(out=fi[:], in_=mi[:, 0:1])
        # px/py assembly (before refinement, as the reference does)
        line = sp.tile([gp, 1], F32, tag="lineg")
        nc.vector.tensor_scalar(
            out=line[:], in0=fi[:], scalar1=float(W), scalar2=None, op0=OP.is_ge,
        )
        px = sp.tile([gp, 1], F32, tag="pxg")
        nc.vector.scalar_tensor_tensor(
            out=px[:], in0=line[:], scalar=-float(W), in1=fi[:],
            op0=OP.mult, op1=OP.add,
        )
        py = sp.tile([gp, 1], F32, tag="pyg")
        nc.vector.scalar_tensor_tensor(
            out=py[:], in0=sv[:], scalar=2.0, in1=line[:],
            op0=OP.mult, op1=OP.add,
        )
        mkp = sp.tile([gp, 1], F32, tag="mkpg")
        nc.vector.tensor_scalar(
            out=mkp[:], in0=scores[:], scalar1=0.0, scalar2=None, op0=OP.is_gt,
        )
        O = sp.tile([gp, 3], F32, tag="Og")
        nc.vector.tensor_tensor(out=O[:, 0:1], in0=px[:], in1=mkp[:], op=OP.mult)
        nc.vector.tensor_tensor(out=O[:, 1:2], in0=py[:], in1=mkp[:], op=OP.mult)
        ilo = sp.tile([gp, 2], F32, tag="ilog")
        nc.vector.tensor_scalar(
            out=ilo[:], in0=O[:, 0:2], scalar1=0.0, scalar2=None, op0=OP.is_gt,
        )
        ihi = sp.tile([gp, 2], F32, tag="ihig")
        nc.vector.tensor_tensor(out=ihi[:], in0=O[:, 0:2], in1=hi2[0:gp], op=OP.is_lt)
        nc.vector.tensor_tensor(out=ilo[:], in0=ilo[:], in1=ihi[:], op=OP.mult)
        intr = sp.tile([gp, 1], F32, tag="intr_g")
        nc.vector.tensor_reduce(out=intr[:], in_=ilo[:], axis=AX.X, op=OP.min)
        # shifted differences over the pair region, then one fused
        # one-hot-select+accumulate per direction
        DH = sp.tile([gp, SW], F32, tag="DHg")
        nc.vector.tensor_tensor(
            out=DH[:], in0=win[:, W + 1 : W + SW + 1],
            in1=win[:, W - 1 : W + SW - 1], op=OP.subtract,
        )
        DV = sp.tile([gp, SW], F32, tag="DVg")
        nc.vector.tensor_tensor(
            out=DV[:], in0=win[:, SW : SW + SW], in1=win[:, 0:SW], op=OP.subtract,
        )
        D2 = sp.tile([gp, 2], F32, tag="D2g")
        junkH = sp.tile([gp, SW], F32, tag="junkH")
        nc.vector.scalar_tensor_tensor(
            out=junkH[:], in0=iot[:], scalar=fi[:, 0:1], in1=DH[:],
            op0=OP.is_equal, op1=OP.mult, accum_out=D2[:, 0:1],
        )
        junkV = sp.tile([gp, SW], F32, tag="junkV")
        nc.vector.scalar_tensor_tensor(
            out=junkV[:], in0=iot[:], scalar=fi[:, 0:1], in1=DV[:],
            op0=OP.is_equal, op1=OP.mult, accum_out=D2[:, 1:2],
        )
        # D = sign(D2) * 0.25 * interior
        Dg = sp.tile([gp, 2], F32, tag="Dg")
        DLg = sp.tile([gp, 2], F32, tag="DLg")
        nc.vector.tensor_scalar(
            out=Dg[:], in0=D2[:], scalar1=0.0, scalar2=None, op0=OP.is_gt,
        )
        nc.vector.tensor_scalar(
            out=DLg[:], in0=D2[:], scalar1=0.0, scalar2=None, op0=OP.is_lt,
        )
        nc.vector.tensor_tensor(out=Dg[:], in0=Dg[:], in1=DLg[:], op=OP.subtract)
        nc.vector.tensor_scalar(
            out=Dg[:], in0=Dg[:], scalar1=0.25, scalar2=None, op0=OP.mult
        )
        nc.vector.tensor_tensor(
            out=Dg[:], in0=Dg[:], in1=intr[:].to_broadcast([gp, 2]), op=OP.mult
        )
        nc.vector.tensor_tensor(out=O[:, 0:2], in0=O[:, 0:2], in1=Dg[:], op=OP.add)
        nc.vector.tensor_copy(out=O[:, 2:3], in_=scores[:])
        nc.sync.dma_start(out=oh[0 : c.P], in_=O[:], single_packet=True)

    nc.compile()
    return nc


def shard_inputs(cfg: Cfg, x: np.ndarray):
    c = cfg
    in_maps = []
    for k in range(c.ncores):
        shard = np.ascontiguousarray(
            x[k * c.BP : (k + 1) * c.BP], dtype=np.float32
        ).reshape(-1)
        xp = np.zeros(c.NPAD, np.float32)
        xp[c.FRONT : c.FRONT + c.SHN] = shard
        in_maps.append({"x": xp})
    return in_maps


def assemble_out(cfg: Cfg, per_core_outs):
    c = cfg
    outs = [o.reshape(c.BP, c.C, 3).astype(np.float32) for o in per_core_outs]
    return np.concatenate(outs, axis=0)


_PROGRAM = None


def _program():
    global _PROGRAM
    if _PROGRAM is None:
        _PROGRAM = build_program(CFG)
    return _PROGRAM


def kernel(x: np.ndarray) -> np.ndarray:
    from concourse.bass_utils import run_bass_kernel_spmd

    c = CFG
    assert x.shape == (c.B, c.C, c.H, c.W), x.shape
    nc = _program()
    in_maps = shard_inputs(c, np.asarray(x))
    res = run_bass_kernel_spmd(nc, in_maps, core_ids=list(range(c.ncores)))
    return assemble_out(c, [res.results[k]["out"] for k in range(c.ncores)])
